# revision 14
# baseline (speedup 1.0000x reference)
"""ABCNN1 attention kernel for 8 Trainium2 NeuronCores.

Reference computation (per batch b of 64, with L=512, D=1024):
    S  = X1 @ X2^T                          (512 x 512)
    A  = S / (|X1_rows| outer |X2_rows|)    cosine match-score
    a1 = A @ W1            a2 = A^T @ W2    (512 x 1024 each)
    attn1 = concat([x1, a1], axis=1)        attn2 = concat([x2, a2], axis=1)

Device strategy (data-parallel, 8 batches per core, no collectives):
  - Host pre-transposes x1/x2 to [b, D, L] so the contraction dim d lands
    on SBUF partitions with fully contiguous DMA.
  - Norms: nsq[l] = sum_d X^2 computed as ones^T @ (X.^2) row-vector
    matmuls, then sqrt + reciprocal; a 4KB DRAM bounce converts the
    [1, 1024] row into per-partition [128, 8] scale columns.
  - Normalization is folded into per-partition scalar multiplies:
      a2's lhsT = diag(r1) @ S   (scaled PSUM->SBUF copy)
      a1's lhsT = diag(r2) @ S^T (scaled PSUM->SBUF copy)
    and the remaining diagonal lands on the matmul *outputs* (also a
    per-partition scaled copy). No cross-partition broadcasts anywhere.
  - Matmuls run in float32r (full-rate on TensorE at N=512, ~1e-4 rel).
  - The concat halves of the outputs are just the inputs; they are
    assembled on host - the device computes and writes only a1/a2.
"""

import numpy as np

B, L, D = 64, 512, 1024
N_CORES = 8
BB = B // N_CORES        # batches per core
KT = D // 128            # contraction tiles (d)
LT = L // 128            # row tiles (l or m)
NT = D // 512            # output free-dim chunks

_CACHE = {}


def _build(bb):
    import concourse.mybir as mybir
    import concourse.tile as tile
    from concourse import bacc
    from concourse import masks

    F32 = mybir.dt.float32
    F32R = mybir.dt.float32r
    BF16 = mybir.dt.bfloat16

    nc = bacc.Bacc("TRN2", target_bir_lowering=False, debug=False,
                   num_devices=N_CORES)
    x1t = nc.declare_dram_parameter("x1t", [bb, D, L], BF16, isOutput=False)
    x2t = nc.declare_dram_parameter("x2t", [bb, D, L], BF16, isOutput=False)
    w1 = nc.declare_dram_parameter("w1", [L, D], F32, isOutput=False)
    w2 = nc.declare_dram_parameter("w2", [L, D], F32, isOutput=False)
    out1 = nc.declare_dram_parameter("out1", [bb, L, D], BF16, isOutput=True)
    out2 = nc.declare_dram_parameter("out2", [bb, L, D], BF16, isOutput=True)

    with tile.TileContext(nc) as tc:
        with (
            tc.tile_pool(name="const", bufs=1) as constp,
            tc.tile_pool(name="xin", bufs=2) as xin,
            tc.tile_pool(name="sq", bufs=2) as sqp,
            tc.tile_pool(name="alhs", bufs=2) as alhsp,
            tc.tile_pool(name="aout", bufs=4) as aoutp,
            tc.tile_pool(name="small", bufs=2) as smallp,
            tc.tile_pool(name="dram", bufs=2, space="DRAM") as dramp,
            tc.tile_pool(name="ps_s", bufs=2, space="PSUM") as ps_s,
            tc.tile_pool(name="ps_nsq", bufs=1, space="PSUM") as ps_nsq,
            tc.tile_pool(name="ps_a", bufs=4, space="PSUM") as ps_a,
        ):
            Copy = mybir.ActivationFunctionType.Copy

            # ---- persistent tiles -------------------------------------
            w1_sb = constp.tile([128, LT, D], BF16, tag="w1")
            w2_sb = constp.tile([128, LT, D], BF16, tag="w2")
            for j in range(LT):
                nc.gpsimd.dma_start(
                    w1_sb[:, j, :], w1[128 * j:128 * (j + 1), :])
                nc.gpsimd.dma_start(
                    w2_sb[:, j, :], w2[128 * j:128 * (j + 1), :])
            ones_sb = constp.tile([128, 1], BF16, tag="ones")
            nc.gpsimd.memset(ones_sb[:], 1.0)
            ident_sb = constp.tile([128, 128], BF16, tag="ident")
            masks.make_identity(nc, ident_sb[:])

            def emit_load_sq(b):
                """Input DMAs (per k-slice) + squares/add-tree -> xsq accums."""
                x1r = xin.tile([128, KT, L], BF16, tag="x1", name="x1r")
                x2r = xin.tile([128, KT, L], BF16, tag="x2", name="x2r")
                for k in range(KT):
                    nc.sync.dma_start(x1r[:, k, :], x1t[b, 128 * k:128 * (k + 1), :])
                    nc.sync.dma_start(x2r[:, k, :], x2t[b, 128 * k:128 * (k + 1), :])

                accs = []
                for xi, x_r in enumerate((x1r, x2r)):
                    xsq = [sqp.tile([128, L], BF16, tag=f"xsq{xi}_{k}",
                                    name=f"xsq{xi}_{k}") for k in range(KT)]
                    for k in range(KT):
                        nc.scalar.square(xsq[k][:], x_r[:, k, :])
                    accs.append(xsq)
                return x1r, x2r, accs

            def emit_norm_tail(accs):
                """ones^T @ xsq_acc matmuls -> sqrt -> DRAM-bounce scatter ->
                reciprocal; returns the per-partition [128, 8] scale tile."""
                nsq1 = ps_nsq.tile([1, L], F32, tag="nsq1", name="nsq1")
                nsq2 = ps_nsq.tile([1, L], F32, tag="nsq2", name="nsq2")
                for xsq in accs:
                    step = 1
                    while step < KT:
                        for k in range(0, KT, 2 * step):
                            nc.vector.tensor_add(xsq[k][:], xsq[k][:], xsq[k + step][:])
                        step *= 2
                nc.tensor.matmul(nsq1[:], ones_sb[:], accs[0][0][:], start=True, stop=True)
                nc.tensor.matmul(nsq2[:], ones_sb[:], accs[1][0][:], start=True, stop=True)
                srow = smallp.tile([1, 2 * L], F32, tag="srow", name="srow")
                nc.scalar.sqrt(srow[:, 0:L], nsq1[:])
                nc.scalar.sqrt(srow[:, L:2 * L], nsq2[:])
                r_dram = dramp.tile([1, 2 * L], F32, tag="rd", name="r_dram")
                nc.sync.dma_start(r_dram[:], srow[:])
                rst_sb = smallp.tile([128, 2 * LT], F32, tag="rst", name="rst_sb")
                nc.sync.dma_start(rst_sb[:], r_dram.rearrange("o (c p) -> (o p) c", p=128))
                r_sb = smallp.tile([128, 2 * LT], F32, tag="rsb", name="r_sb")
                nc.vector.reciprocal(r_sb[:], rst_sb[:])
                return r_sb

            def emit_s_matmuls(x1r, x2r, r_sb):
                """S with folded r1 scaling -> a2lhs; a1lhs = PE-transpose of
                a2lhs (so a1lhs = S^T D1^-1); W1n = r2-scaled W1 compensates.

                a2lhs = D1^-1 S:  a2 = (a2lhs)^T W2 = S^T D1^-1 W2, r2-scaled rows.
                a1lhs = (a2lhs)^T: a1 = (a1lhs)^T W1n = D1^-1 S D2^-1 W1 exactly.
                """
                a2lhs = alhsp.tile([128, LT, L], BF16, tag="a2lhs", name="a2lhs")
                a1lhs = alhsp.tile([128, LT, L], BF16, tag="a1lhs", name="a1lhs")
                for i in range(LT):
                    s_ps = ps_s.tile([128, L], F32, tag="s", name="s_ps")
                    for k in range(KT):
                        nc.tensor.matmul(s_ps[:], x1r[:, k, 128 * i:128 * (i + 1)],
                                         x2r[:, k, :], start=(k == 0), stop=(k == KT - 1))
                    nc.vector.tensor_scalar_mul(a2lhs[:, i, :], s_ps[:], r_sb[:, i:i + 1])
                for jp in range(LT // 2):
                    t_ps = ps_a.tile([128, 2 * L], BF16, tag="a", name="t_ps")
                    for jj in range(2):
                        j = 2 * jp + jj
                        for i in range(LT):
                            nc.tensor.transpose(
                                t_ps[:, 512 * jj + 128 * i:512 * jj + 128 * (i + 1)],
                                a2lhs[:, i, 128 * j:128 * (j + 1)], ident_sb[:])
                        nc.vector.tensor_copy(a1lhs[:, 2 * jp + jj, :],
                                              t_ps[:, 512 * jj:512 * (jj + 1)])
                # per-batch W1n = diag(r2) @ W1 (rows m scaled by r2)
                w1n_sb = alhsp.tile([128, LT, D], BF16, tag="w1n", name="w1n_sb")
                for j in range(LT):
                    nc.vector.tensor_scalar_mul(w1n_sb[:, j, :], w1_sb[:, j, :],
                                                r_sb[:, LT + j:LT + j + 1])
                return a1lhs, a2lhs, w1n_sb

            def emit_stage2(b, a1lhs, a2lhs, w1n_sb, r_sb):
                # stage 2 (bf16): a1 = (S D2^-1 W1) row-scaled by r1,
                #                 a2 = (S^T D1^-1 W2) row-scaled by r2
                for i in range(LT):
                    a1_sb = aoutp.tile([128, D], BF16, tag="aout", name="a1_sb")
                    for n in range(NT):
                        a1_ps = ps_a.tile([128, 512], F32, tag="a", name="a1_ps")
                        for jj in range(LT):
                            nc.tensor.matmul(
                                a1_ps[:], a1lhs[:, jj, 128 * i:128 * (i + 1)],
                                w1n_sb[:, jj, 512 * n:512 * (n + 1)],
                                start=(jj == 0), stop=(jj == LT - 1))
                        nc.scalar.copy(a1_sb[:, 512 * n:512 * (n + 1)], a1_ps[:])
                    nc.gpsimd.dma_start(out1[b, 128 * i:128 * (i + 1), :], a1_sb[:])
                for j in range(LT):
                    a2_sb = aoutp.tile([128, D], BF16, tag="aout", name="a2_sb")
                    for n in range(NT):
                        a2_ps = ps_a.tile([128, 512], F32, tag="a", name="a2_ps")
                        for ii in range(LT):
                            nc.tensor.matmul(
                                a2_ps[:], a2lhs[:, ii, 128 * j:128 * (j + 1)],
                                w2_sb[:, ii, 512 * n:512 * (n + 1)],
                                start=(ii == 0), stop=(ii == LT - 1))
                        nc.vector.tensor_scalar_mul(a2_sb[:, 512 * n:512 * (n + 1)],
                                                    a2_ps[:], r_sb[:, LT + j:LT + j + 1])
                    nc.gpsimd.dma_start(out2[b, 128 * j:128 * (j + 1), :], a2_sb[:])

            # Software pipeline. Batch b's loads + squares land a batch
            # early; its norm tail (nsq matmuls + scatter + reciprocal) is
            # emitted between batch b-1's S-matmuls and stage 2, so the PE
            # stream never head-of-line blocks on the norm latency chain
            # and r_sb is ready before batch b's PSUM copies need it.
            x1r, x2r, accs = emit_load_sq(0)
            r_sb = emit_norm_tail(accs)
            prev = None  # (b, a1lhs, a2lhs, r_sb) awaiting stage 2
            for b in range(bb):
                if b + 1 < bb:
                    nxt = emit_load_sq(b + 1)
                else:
                    nxt = None
                a1lhs, a2lhs, w1n_sb = emit_s_matmuls(x1r, x2r, r_sb)
                if prev is not None:
                    emit_stage2(*prev)
                prev = (b, a1lhs, a2lhs, w1n_sb, r_sb)
                if nxt is not None:
                    x1r, x2r, accs = nxt
                    r_sb = emit_norm_tail(accs)
            emit_stage2(*prev)

    nc.compile()
    return nc


def _get_nc(bb=BB):
    if bb not in _CACHE:
        _CACHE[bb] = _build(bb)
    return _CACHE[bb]


def run_device(x1, x2, W1, W2, trace=False, bb=BB, n_batches=None):
    """Run the device part; returns (a1, a2) of shape (n, L, D) and the
    raw BassKernelResults (for exec_time_ns when trace=True)."""
    import concourse.bass_utils as bass_utils

    import ml_dtypes
    bf16 = ml_dtypes.bfloat16
    n = n_batches if n_batches is not None else bb * N_CORES
    x1 = np.asarray(x1, dtype=np.float32).reshape(n, L, D).transpose(0, 2, 1).astype(bf16)
    x2 = np.asarray(x2, dtype=np.float32).reshape(n, L, D).transpose(0, 2, 1).astype(bf16)
    W1 = np.ascontiguousarray(np.asarray(W1, dtype=np.float32))
    W2 = np.ascontiguousarray(np.asarray(W2, dtype=np.float32))

    nc = _get_nc(bb)
    in_maps = []
    for c in range(N_CORES):
        s = slice(c * bb, (c + 1) * bb)
        in_maps.append({"x1t": x1[s], "x2t": x2[s], "w1": W1, "w2": W2})
    res = bass_utils.run_bass_kernel_spmd(nc, in_maps, list(range(N_CORES)),
                                          trace=trace)
    a1 = np.concatenate([res.results[c]["out1"].astype(np.float32)
                         for c in range(N_CORES)], axis=0)
    a2 = np.concatenate([res.results[c]["out2"].astype(np.float32)
                         for c in range(N_CORES)], axis=0)
    return a1, a2, res


def kernel(x1, x2, W1, W2):
    x1 = np.asarray(x1, dtype=np.float32)
    x2 = np.asarray(x2, dtype=np.float32)
    a1, a2, _ = run_device(x1, x2, W1, W2, trace=False)
    attn1 = np.stack([x1.reshape(B, L, D), a1], axis=1)
    attn2 = np.stack([x2.reshape(B, L, D), a2], axis=1)
    return attn1, attn2


# revision 15
# speedup vs baseline: 1.2028x; 1.2028x over previous
"""ABCNN1 attention kernel for 8 Trainium2 NeuronCores.

Reference computation (per batch b of 64, with L=512, D=1024):
    S  = X1 @ X2^T                          (512 x 512)
    A  = S / (|X1_rows| outer |X2_rows|)    cosine match-score
    a1 = A @ W1            a2 = A^T @ W2    (512 x 1024 each)
    attn1 = concat([x1, a1], axis=1)        attn2 = concat([x2, a2], axis=1)

Device strategy (data-parallel, 8 batches per core, no collectives):
  - Host pre-transposes x1/x2 to [b, D, L] so the contraction dim d lands
    on SBUF partitions with fully contiguous DMA.
  - Norms: nsq[l] = sum_d X^2 computed as ones^T @ (X.^2) row-vector
    matmuls, then sqrt + reciprocal; a 4KB DRAM bounce converts the
    [1, 1024] row into per-partition [128, 8] scale columns.
  - Normalization is folded into per-partition scalar multiplies:
      a2's lhsT = diag(r1) @ S   (scaled PSUM->SBUF copy)
      a1's lhsT = diag(r2) @ S^T (scaled PSUM->SBUF copy)
    and the remaining diagonal lands on the matmul *outputs* (also a
    per-partition scaled copy). No cross-partition broadcasts anywhere.
  - Matmuls run in float32r (full-rate on TensorE at N=512, ~1e-4 rel).
  - The concat halves of the outputs are just the inputs; they are
    assembled on host - the device computes and writes only a1/a2.
"""

import numpy as np

B, L, D = 64, 512, 1024
N_CORES = 8
BB = B // N_CORES        # batches per core
KT = D // 128            # contraction tiles (d)
LT = L // 128            # row tiles (l or m)
NT = D // 512            # output free-dim chunks

_CACHE = {}


def _build(bb):
    import concourse.mybir as mybir
    import concourse.tile as tile
    from concourse import bacc
    from concourse import masks

    F32 = mybir.dt.float32
    F32R = mybir.dt.float32r
    BF16 = mybir.dt.bfloat16

    nc = bacc.Bacc("TRN2", target_bir_lowering=False, debug=False,
                   num_devices=N_CORES)
    x1t = nc.declare_dram_parameter("x1t", [bb, D, L], BF16, isOutput=False)
    x2t = nc.declare_dram_parameter("x2t", [bb, D, L], BF16, isOutput=False)
    w1 = nc.declare_dram_parameter("w1", [L, D], F32, isOutput=False)
    w2 = nc.declare_dram_parameter("w2", [L, D], F32, isOutput=False)
    out1 = nc.declare_dram_parameter("out1", [bb, L, D], BF16, isOutput=True)
    out2 = nc.declare_dram_parameter("out2", [bb, L, D], BF16, isOutput=True)

    with tile.TileContext(nc) as tc:
        with (
            tc.tile_pool(name="const", bufs=1) as constp,
            tc.tile_pool(name="xin", bufs=2) as xin,
            tc.tile_pool(name="sq", bufs=2) as sqp,
            tc.tile_pool(name="alhs", bufs=2) as alhsp,
            tc.tile_pool(name="aout", bufs=4) as aoutp,
            tc.tile_pool(name="small", bufs=2) as smallp,
            tc.tile_pool(name="dram", bufs=2, space="DRAM") as dramp,
            tc.tile_pool(name="ps_s", bufs=2, space="PSUM") as ps_s,
            tc.tile_pool(name="ps_t", bufs=1, space="PSUM") as ps_t,
            tc.tile_pool(name="ps_nsq", bufs=1, space="PSUM") as ps_nsq,
            tc.tile_pool(name="ps_a", bufs=3, space="PSUM") as ps_a,
        ):
            Copy = mybir.ActivationFunctionType.Copy

            # ---- persistent tiles -------------------------------------
            w1_sb = constp.tile([128, LT, D], BF16, tag="w1")
            w2_sb = constp.tile([128, LT, D], BF16, tag="w2")
            for j in range(LT):
                nc.gpsimd.dma_start(
                    w1_sb[:, j, :], w1[128 * j:128 * (j + 1), :])
                nc.gpsimd.dma_start(
                    w2_sb[:, j, :], w2[128 * j:128 * (j + 1), :])
            ones_sb = constp.tile([128, 1], BF16, tag="ones")
            nc.gpsimd.memset(ones_sb[:], 1.0)
            ident_sb = constp.tile([128, 128], BF16, tag="ident")
            masks.make_identity(nc, ident_sb[:])

            def emit_load_sq(b):
                """Input DMAs (per k-slice) + squares/add-tree -> xsq accums."""
                x1r = xin.tile([128, KT, L], BF16, tag="x1", name="x1r")
                x2r = xin.tile([128, KT, L], BF16, tag="x2", name="x2r")
                for k in range(KT):
                    nc.sync.dma_start(x1r[:, k, :], x1t[b, 128 * k:128 * (k + 1), :])
                    nc.sync.dma_start(x2r[:, k, :], x2t[b, 128 * k:128 * (k + 1), :])

                accs = []
                for xi, x_r in enumerate((x1r, x2r)):
                    xsq = [sqp.tile([128, L], BF16, tag=f"xsq{xi}_{k}",
                                    name=f"xsq{xi}_{k}") for k in range(KT)]
                    for k in range(KT):
                        nc.scalar.square(xsq[k][:], x_r[:, k, :])
                    accs.append(xsq)
                return x1r, x2r, accs

            def emit_norm_tail(accs):
                """ones^T @ xsq_acc matmuls -> sqrt -> DRAM-bounce scatter ->
                reciprocal; returns the per-partition [128, 8] scale tile."""
                nsq1 = ps_nsq.tile([1, L], F32, tag="nsq1", name="nsq1")
                nsq2 = ps_nsq.tile([1, L], F32, tag="nsq2", name="nsq2")
                for xsq in accs:
                    step = 1
                    while step < KT:
                        for k in range(0, KT, 2 * step):
                            nc.vector.tensor_add(xsq[k][:], xsq[k][:], xsq[k + step][:])
                        step *= 2
                nc.tensor.matmul(nsq1[:], ones_sb[:], accs[0][0][:], start=True, stop=True)
                nc.tensor.matmul(nsq2[:], ones_sb[:], accs[1][0][:], start=True, stop=True)
                srow = smallp.tile([1, 2 * L], F32, tag="srow", name="srow")
                nc.scalar.sqrt(srow[:, 0:L], nsq1[:])
                nc.scalar.sqrt(srow[:, L:2 * L], nsq2[:])
                r_dram = dramp.tile([1, 2 * L], F32, tag="rd", name="r_dram")
                nc.sync.dma_start(r_dram[:], srow[:])
                rst_sb = smallp.tile([128, 2 * LT], F32, tag="rst", name="rst_sb")
                nc.sync.dma_start(rst_sb[:], r_dram.rearrange("o (c p) -> (o p) c", p=128))
                r_sb = smallp.tile([128, 2 * LT], F32, tag="rsb", name="r_sb")
                nc.vector.reciprocal(r_sb[:], rst_sb[:])
                return r_sb

            def emit_s_matmuls(x1r, x2r, r_sb):
                """S with folded r1 scaling -> a2lhs; a1lhs = PE-transpose of
                a2lhs (so a1lhs = S^T D1^-1); W1n = r2-scaled W1 compensates.

                a2lhs = D1^-1 S:  a2 = (a2lhs)^T W2 = S^T D1^-1 W2, r2-scaled rows.
                a1lhs = (a2lhs)^T: a1 = (a1lhs)^T W1n = D1^-1 S D2^-1 W1 exactly.
                """
                a2lhs = alhsp.tile([128, LT, L], BF16, tag="a2lhs", name="a2lhs")
                a1lhs = alhsp.tile([128, LT, L], BF16, tag="a1lhs", name="a1lhs")
                for i in range(LT):
                    s_ps = ps_s.tile([128, L], F32, tag="s", name="s_ps")
                    for k in range(KT):
                        nc.tensor.matmul(s_ps[:], x1r[:, k, 128 * i:128 * (i + 1)],
                                         x2r[:, k, :], start=(k == 0), stop=(k == KT - 1))
                    nc.vector.tensor_scalar_mul(a2lhs[:, i, :], s_ps[:], r_sb[:, i:i + 1])
                for jp in range(LT // 2):
                    t_ps = ps_t.tile([128, 2 * L], BF16, tag="t", name="t_ps")
                    for jj in range(2):
                        j = 2 * jp + jj
                        for i in range(LT):
                            nc.tensor.transpose(
                                t_ps[:, 512 * jj + 128 * i:512 * jj + 128 * (i + 1)],
                                a2lhs[:, i, 128 * j:128 * (j + 1)], ident_sb[:])
                        nc.vector.tensor_copy(a1lhs[:, 2 * jp + jj, :],
                                              t_ps[:, 512 * jj:512 * (jj + 1)])
                # per-batch W1n = diag(r2) @ W1 (rows m scaled by r2)
                w1n_sb = alhsp.tile([128, LT, D], BF16, tag="w1n", name="w1n_sb")
                for j in range(LT):
                    nc.vector.tensor_scalar_mul(w1n_sb[:, j, :], w1_sb[:, j, :],
                                                r_sb[:, LT + j:LT + j + 1])
                return a1lhs, a2lhs, w1n_sb

            def emit_stage2(b, a1lhs, a2lhs, w1n_sb, r_sb):
                # stage 2 (bf16): a1 = (S D2^-1 W1) row-scaled by r1,
                #                 a2 = (S^T D1^-1 W2) row-scaled by r2
                for i in range(LT):
                    a1_sb = aoutp.tile([128, D], BF16, tag="aout", name="a1_sb")
                    for n in range(NT):
                        a1_ps = ps_a.tile([128, 512], F32, tag="a", name="a1_ps")
                        for jj in range(LT):
                            nc.tensor.matmul(
                                a1_ps[:], a1lhs[:, jj, 128 * i:128 * (i + 1)],
                                w1n_sb[:, jj, 512 * n:512 * (n + 1)],
                                start=(jj == 0), stop=(jj == LT - 1))
                        nc.scalar.copy(a1_sb[:, 512 * n:512 * (n + 1)], a1_ps[:])
                    nc.gpsimd.dma_start(out1[b, 128 * i:128 * (i + 1), :], a1_sb[:])
                for j in range(LT):
                    a2_sb = aoutp.tile([128, D], BF16, tag="aout", name="a2_sb")
                    for n in range(NT):
                        a2_ps = ps_a.tile([128, 512], F32, tag="a", name="a2_ps")
                        for ii in range(LT):
                            nc.tensor.matmul(
                                a2_ps[:], a2lhs[:, ii, 128 * j:128 * (j + 1)],
                                w2_sb[:, ii, 512 * n:512 * (n + 1)],
                                start=(ii == 0), stop=(ii == LT - 1))
                        nc.vector.tensor_scalar_mul(a2_sb[:, 512 * n:512 * (n + 1)],
                                                    a2_ps[:], r_sb[:, LT + j:LT + j + 1])
                    nc.gpsimd.dma_start(out2[b, 128 * j:128 * (j + 1), :], a2_sb[:])

            # Software pipeline. Batch b's loads + squares land a batch
            # early; its norm tail (nsq matmuls + scatter + reciprocal) is
            # emitted between batch b-1's S-matmuls and stage 2, so the PE
            # stream never head-of-line blocks on the norm latency chain
            # and r_sb is ready before batch b's PSUM copies need it.
            x1r, x2r, accs = emit_load_sq(0)
            r_sb = emit_norm_tail(accs)
            prev = None  # (b, a1lhs, a2lhs, r_sb) awaiting stage 2
            for b in range(bb):
                if b + 1 < bb:
                    nxt = emit_load_sq(b + 1)
                else:
                    nxt = None
                a1lhs, a2lhs, w1n_sb = emit_s_matmuls(x1r, x2r, r_sb)
                if prev is not None:
                    emit_stage2(*prev)
                prev = (b, a1lhs, a2lhs, w1n_sb, r_sb)
                if nxt is not None:
                    x1r, x2r, accs = nxt
                    r_sb = emit_norm_tail(accs)
            emit_stage2(*prev)

    nc.compile()
    return nc


def _get_nc(bb=BB):
    if bb not in _CACHE:
        _CACHE[bb] = _build(bb)
    return _CACHE[bb]


def run_device(x1, x2, W1, W2, trace=False, bb=BB, n_batches=None):
    """Run the device part; returns (a1, a2) of shape (n, L, D) and the
    raw BassKernelResults (for exec_time_ns when trace=True)."""
    import concourse.bass_utils as bass_utils

    import ml_dtypes
    bf16 = ml_dtypes.bfloat16
    n = n_batches if n_batches is not None else bb * N_CORES
    x1 = np.asarray(x1, dtype=np.float32).reshape(n, L, D).transpose(0, 2, 1).astype(bf16)
    x2 = np.asarray(x2, dtype=np.float32).reshape(n, L, D).transpose(0, 2, 1).astype(bf16)
    W1 = np.ascontiguousarray(np.asarray(W1, dtype=np.float32))
    W2 = np.ascontiguousarray(np.asarray(W2, dtype=np.float32))

    nc = _get_nc(bb)
    in_maps = []
    for c in range(N_CORES):
        s = slice(c * bb, (c + 1) * bb)
        in_maps.append({"x1t": x1[s], "x2t": x2[s], "w1": W1, "w2": W2})
    res = bass_utils.run_bass_kernel_spmd(nc, in_maps, list(range(N_CORES)),
                                          trace=trace)
    a1 = np.concatenate([res.results[c]["out1"].astype(np.float32)
                         for c in range(N_CORES)], axis=0)
    a2 = np.concatenate([res.results[c]["out2"].astype(np.float32)
                         for c in range(N_CORES)], axis=0)
    return a1, a2, res


def kernel(x1, x2, W1, W2):
    x1 = np.asarray(x1, dtype=np.float32)
    x2 = np.asarray(x2, dtype=np.float32)
    a1, a2, _ = run_device(x1, x2, W1, W2, trace=False)
    attn1 = np.stack([x1.reshape(B, L, D), a1], axis=1)
    attn2 = np.stack([x2.reshape(B, L, D), a2], axis=1)
    return attn1, attn2


# revision 16
# speedup vs baseline: 1.2213x; 1.0154x over previous
"""ABCNN1 attention kernel for 8 Trainium2 NeuronCores.

Reference computation (per batch b of 64, with L=512, D=1024):
    S  = X1 @ X2^T                          (512 x 512)
    A  = S / (|X1_rows| outer |X2_rows|)    cosine match-score
    a1 = A @ W1            a2 = A^T @ W2    (512 x 1024 each)
    attn1 = concat([x1, a1], axis=1)        attn2 = concat([x2, a2], axis=1)

Device strategy (data-parallel, 8 batches per core, no collectives):
  - Host pre-transposes x1/x2 to [b, D, L] so the contraction dim d lands
    on SBUF partitions with fully contiguous DMA.
  - Norms: nsq[l] = sum_d X^2 computed as ones^T @ (X.^2) row-vector
    matmuls, then sqrt + reciprocal; a 4KB DRAM bounce converts the
    [1, 1024] row into per-partition [128, 8] scale columns.
  - Normalization is folded into per-partition scalar multiplies:
      a2's lhsT = diag(r1) @ S   (scaled PSUM->SBUF copy)
      a1's lhsT = diag(r2) @ S^T (scaled PSUM->SBUF copy)
    and the remaining diagonal lands on the matmul *outputs* (also a
    per-partition scaled copy). No cross-partition broadcasts anywhere.
  - Matmuls run in float32r (full-rate on TensorE at N=512, ~1e-4 rel).
  - The concat halves of the outputs are just the inputs; they are
    assembled on host - the device computes and writes only a1/a2.
"""

import numpy as np

B, L, D = 64, 512, 1024
N_CORES = 8
BB = B // N_CORES        # batches per core
KT = D // 128            # contraction tiles (d)
LT = L // 128            # row tiles (l or m)
NT = D // 512            # output free-dim chunks

_CACHE = {}


def _build(bb):
    import concourse.mybir as mybir
    import concourse.tile as tile
    from concourse import bacc
    from concourse import masks

    F32 = mybir.dt.float32
    F32R = mybir.dt.float32r
    BF16 = mybir.dt.bfloat16

    nc = bacc.Bacc("TRN2", target_bir_lowering=False, debug=False,
                   num_devices=N_CORES)
    x1t = nc.declare_dram_parameter("x1t", [bb, D, L], BF16, isOutput=False)
    x2t = nc.declare_dram_parameter("x2t", [bb, D, L], BF16, isOutput=False)
    w1 = nc.declare_dram_parameter("w1", [L, D], F32, isOutput=False)
    w2 = nc.declare_dram_parameter("w2", [L, D], F32, isOutput=False)
    out1 = nc.declare_dram_parameter("out1", [bb, L, D], BF16, isOutput=True)
    out2 = nc.declare_dram_parameter("out2", [bb, L, D], BF16, isOutput=True)

    with tile.TileContext(nc) as tc:
        with (
            tc.tile_pool(name="const", bufs=1) as constp,
            tc.tile_pool(name="xin", bufs=2) as xin,
            tc.tile_pool(name="sq", bufs=2) as sqp,
            tc.tile_pool(name="alhs", bufs=2) as alhsp,
            tc.tile_pool(name="aout", bufs=4) as aoutp,
            tc.tile_pool(name="small", bufs=2) as smallp,
            tc.tile_pool(name="dram", bufs=2, space="DRAM") as dramp,
            tc.tile_pool(name="ps_s", bufs=3, space="PSUM") as ps_s,
            tc.tile_pool(name="ps_t", bufs=1, space="PSUM") as ps_t,
            tc.tile_pool(name="ps_nsq", bufs=1, space="PSUM") as ps_nsq,
            tc.tile_pool(name="ps_a", bufs=2, space="PSUM") as ps_a,
        ):
            Copy = mybir.ActivationFunctionType.Copy

            # ---- persistent tiles -------------------------------------
            w1_sb = constp.tile([128, LT, D], BF16, tag="w1")
            w2_sb = constp.tile([128, LT, D], BF16, tag="w2")
            for j in range(LT):
                nc.gpsimd.dma_start(
                    w1_sb[:, j, :], w1[128 * j:128 * (j + 1), :])
                nc.gpsimd.dma_start(
                    w2_sb[:, j, :], w2[128 * j:128 * (j + 1), :])
            ones_sb = constp.tile([128, 1], BF16, tag="ones")
            nc.gpsimd.memset(ones_sb[:], 1.0)
            ident_sb = constp.tile([128, 128], BF16, tag="ident")
            masks.make_identity(nc, ident_sb[:])

            def emit_load_sq(b):
                """Input DMAs (per k-slice) + squares/add-tree -> xsq accums."""
                x1r = xin.tile([128, KT, L], BF16, tag="x1", name="x1r")
                x2r = xin.tile([128, KT, L], BF16, tag="x2", name="x2r")
                for k in range(KT):
                    nc.sync.dma_start(x1r[:, k, :], x1t[b, 128 * k:128 * (k + 1), :])
                    nc.sync.dma_start(x2r[:, k, :], x2t[b, 128 * k:128 * (k + 1), :])

                accs = []
                for xi, x_r in enumerate((x1r, x2r)):
                    xsq = [sqp.tile([128, L], BF16, tag=f"xsq{xi}_{k}",
                                    name=f"xsq{xi}_{k}") for k in range(KT)]
                    for k in range(KT):
                        nc.scalar.square(xsq[k][:], x_r[:, k, :])
                    accs.append(xsq)
                return x1r, x2r, accs

            def emit_norm_tail(accs):
                """ones^T @ xsq_acc matmuls -> sqrt -> DRAM-bounce scatter ->
                reciprocal; returns the per-partition [128, 8] scale tile."""
                nsq1 = ps_nsq.tile([1, L], F32, tag="nsq1", name="nsq1")
                nsq2 = ps_nsq.tile([1, L], F32, tag="nsq2", name="nsq2")
                for xsq in accs:
                    step = 1
                    while step < KT:
                        for k in range(0, KT, 2 * step):
                            nc.vector.tensor_add(xsq[k][:], xsq[k][:], xsq[k + step][:])
                        step *= 2
                nc.tensor.matmul(nsq1[:], ones_sb[:], accs[0][0][:], start=True, stop=True)
                nc.tensor.matmul(nsq2[:], ones_sb[:], accs[1][0][:], start=True, stop=True)
                srow = smallp.tile([1, 2 * L], F32, tag="srow", name="srow")
                nc.scalar.sqrt(srow[:, 0:L], nsq1[:])
                nc.scalar.sqrt(srow[:, L:2 * L], nsq2[:])
                r_dram = dramp.tile([1, 2 * L], F32, tag="rd", name="r_dram")
                nc.sync.dma_start(r_dram[:], srow[:])
                rst_sb = smallp.tile([128, 2 * LT], F32, tag="rst", name="rst_sb")
                nc.sync.dma_start(rst_sb[:], r_dram.rearrange("o (c p) -> (o p) c", p=128))
                r_sb = smallp.tile([128, 2 * LT], F32, tag="rsb", name="r_sb")
                nc.vector.reciprocal(r_sb[:], rst_sb[:])
                return r_sb

            def emit_s_matmuls(x1r, x2r, r_sb):
                """S with folded r1 scaling -> a2lhs; a1lhs = PE-transpose of
                a2lhs (so a1lhs = S^T D1^-1); W1n = r2-scaled W1 compensates.

                a2lhs = D1^-1 S:  a2 = (a2lhs)^T W2 = S^T D1^-1 W2, r2-scaled rows.
                a1lhs = (a2lhs)^T: a1 = (a1lhs)^T W1n = D1^-1 S D2^-1 W1 exactly.
                """
                a2lhs = alhsp.tile([128, LT, L], BF16, tag="a2lhs", name="a2lhs")
                a1lhs = alhsp.tile([128, LT, L], BF16, tag="a1lhs", name="a1lhs")
                for i in range(LT):
                    s_ps = ps_s.tile([128, L], F32, tag="s", name="s_ps")
                    for k in range(KT):
                        nc.tensor.matmul(s_ps[:], x1r[:, k, 128 * i:128 * (i + 1)],
                                         x2r[:, k, :], start=(k == 0), stop=(k == KT - 1))
                    nc.vector.tensor_scalar_mul(a2lhs[:, i, :], s_ps[:], r_sb[:, i:i + 1])
                for jp in range(LT // 2):
                    t_ps = ps_t.tile([128, 2 * L], BF16, tag="t", name="t_ps")
                    for jj in range(2):
                        j = 2 * jp + jj
                        for i in range(LT):
                            nc.tensor.transpose(
                                t_ps[:, 512 * jj + 128 * i:512 * jj + 128 * (i + 1)],
                                a2lhs[:, i, 128 * j:128 * (j + 1)], ident_sb[:])
                        nc.vector.tensor_copy(a1lhs[:, 2 * jp + jj, :],
                                              t_ps[:, 512 * jj:512 * (jj + 1)])
                # per-batch W1n = diag(r2) @ W1 (rows m scaled by r2)
                w1n_sb = alhsp.tile([128, LT, D], BF16, tag="w1n", name="w1n_sb")
                for j in range(LT):
                    nc.vector.tensor_scalar_mul(w1n_sb[:, j, :], w1_sb[:, j, :],
                                                r_sb[:, LT + j:LT + j + 1])
                return a1lhs, a2lhs, w1n_sb

            def emit_stage2(b, a1lhs, a2lhs, w1n_sb, r_sb):
                # stage 2 (bf16): a1 = (S D2^-1 W1) row-scaled by r1,
                #                 a2 = (S^T D1^-1 W2) row-scaled by r2
                for i in range(LT):
                    a1_sb = aoutp.tile([128, D], BF16, tag="aout", name="a1_sb")
                    for n in range(NT):
                        a1_ps = ps_a.tile([128, 512], F32, tag="a", name="a1_ps")
                        for jj in range(LT):
                            nc.tensor.matmul(
                                a1_ps[:], a1lhs[:, jj, 128 * i:128 * (i + 1)],
                                w1n_sb[:, jj, 512 * n:512 * (n + 1)],
                                start=(jj == 0), stop=(jj == LT - 1))
                        nc.scalar.copy(a1_sb[:, 512 * n:512 * (n + 1)], a1_ps[:])
                    nc.gpsimd.dma_start(out1[b, 128 * i:128 * (i + 1), :], a1_sb[:])
                for j in range(LT):
                    a2_sb = aoutp.tile([128, D], BF16, tag="aout", name="a2_sb")
                    for n in range(NT):
                        a2_ps = ps_a.tile([128, 512], F32, tag="a", name="a2_ps")
                        for ii in range(LT):
                            nc.tensor.matmul(
                                a2_ps[:], a2lhs[:, ii, 128 * j:128 * (j + 1)],
                                w2_sb[:, ii, 512 * n:512 * (n + 1)],
                                start=(ii == 0), stop=(ii == LT - 1))
                        nc.vector.tensor_scalar_mul(a2_sb[:, 512 * n:512 * (n + 1)],
                                                    a2_ps[:], r_sb[:, LT + j:LT + j + 1])
                    nc.gpsimd.dma_start(out2[b, 128 * j:128 * (j + 1), :], a2_sb[:])

            # Software pipeline. Batch b's loads + squares land a batch
            # early; its norm tail (nsq matmuls + scatter + reciprocal) is
            # emitted between batch b-1's S-matmuls and stage 2, so the PE
            # stream never head-of-line blocks on the norm latency chain
            # and r_sb is ready before batch b's PSUM copies need it.
            x1r, x2r, accs = emit_load_sq(0)
            r_sb = emit_norm_tail(accs)
            prev = None  # (b, a1lhs, a2lhs, r_sb) awaiting stage 2
            for b in range(bb):
                if b + 1 < bb:
                    nxt = emit_load_sq(b + 1)
                else:
                    nxt = None
                a1lhs, a2lhs, w1n_sb = emit_s_matmuls(x1r, x2r, r_sb)
                if prev is not None:
                    emit_stage2(*prev)
                prev = (b, a1lhs, a2lhs, w1n_sb, r_sb)
                if nxt is not None:
                    x1r, x2r, accs = nxt
                    r_sb = emit_norm_tail(accs)
            emit_stage2(*prev)

    nc.compile()
    return nc


def _get_nc(bb=BB):
    if bb not in _CACHE:
        _CACHE[bb] = _build(bb)
    return _CACHE[bb]


def run_device(x1, x2, W1, W2, trace=False, bb=BB, n_batches=None):
    """Run the device part; returns (a1, a2) of shape (n, L, D) and the
    raw BassKernelResults (for exec_time_ns when trace=True)."""
    import concourse.bass_utils as bass_utils

    import ml_dtypes
    bf16 = ml_dtypes.bfloat16
    n = n_batches if n_batches is not None else bb * N_CORES
    x1 = np.asarray(x1, dtype=np.float32).reshape(n, L, D).transpose(0, 2, 1).astype(bf16)
    x2 = np.asarray(x2, dtype=np.float32).reshape(n, L, D).transpose(0, 2, 1).astype(bf16)
    W1 = np.ascontiguousarray(np.asarray(W1, dtype=np.float32))
    W2 = np.ascontiguousarray(np.asarray(W2, dtype=np.float32))

    nc = _get_nc(bb)
    in_maps = []
    for c in range(N_CORES):
        s = slice(c * bb, (c + 1) * bb)
        in_maps.append({"x1t": x1[s], "x2t": x2[s], "w1": W1, "w2": W2})
    res = bass_utils.run_bass_kernel_spmd(nc, in_maps, list(range(N_CORES)),
                                          trace=trace)
    a1 = np.concatenate([res.results[c]["out1"].astype(np.float32)
                         for c in range(N_CORES)], axis=0)
    a2 = np.concatenate([res.results[c]["out2"].astype(np.float32)
                         for c in range(N_CORES)], axis=0)
    return a1, a2, res


def kernel(x1, x2, W1, W2):
    x1 = np.asarray(x1, dtype=np.float32)
    x2 = np.asarray(x2, dtype=np.float32)
    a1, a2, _ = run_device(x1, x2, W1, W2, trace=False)
    attn1 = np.stack([x1.reshape(B, L, D), a1], axis=1)
    attn2 = np.stack([x2.reshape(B, L, D), a2], axis=1)
    return attn1, attn2
